# revision 39
# baseline (speedup 1.0000x reference)
"""Trainium2 Bass kernel for the BNN FASHION FC problem.

Network (per reference):
  h = x.reshape(B, 784)
  L1: h @ binarize(w1).T + b1 -> BN -> clip -> binarize     [B, 2048]
  L2: h @ binarize(w2).T + b2 -> BN -> clip -> binarize     [B, 2048]
  L3: (h @ binarize(w3).T + b3) * scale                     [B, 10]

Strategy (8 NeuronCores, data-parallel over batch, weights replicated):
  - Everything is computed with the hidden index on SBUF/PSUM partitions,
    so batchnorm affine + sign folds into one ScalarE activation (Sign
    with per-partition scale/bias) and layer N's output lands exactly in
    the [k=hidden, free=batch] layout layer N+1 needs.
  - L1 splits x = hi + lo. hi runs in fp16 over 7 k-tiles, with the
    packed tail tile carrying hi rows 0:16 and an exact fp16 lo-tail in
    rows 16:32 (both against +-1 fp16 weights).  lo for the six full
    k-tiles runs in fp8e4m3 scaled by 2^14 against +-2^-14 fp8e5m2
    weights in DoubleRow mode (3 matmuls).  10 matmuls/tile total vs 13
    for the exact fp16 hi/lo scheme; final rel err ~1.4e-2 (gate 2e-2).
  - L2/L3 operands are all +-1: exact in fp8e4, run with DoubleRow.
  - binarize(clip(y)) == binarize(y), so clip is dropped.
  - BN folding: y = mm*inv + c with inv = g/sqrt(v+eps), c = (b-m)*inv + be.
  - L3 DoubleRow matmuls are interleaved into the L2 o-loop (lagged so
    the Sign drains stay off the PE critical path), shortening the tail.

Output per core is [10, 2048] (hidden-major); the host transposes and
concatenates to the full [16384, 10].
"""

import numpy as np
from contextlib import ExitStack

try:
    import concourse.bass as bass
except ImportError:  # staged repo location
    import sys

    sys.path.insert(0, "/opt/trn_rl_repo")
    import concourse.bass as bass

import concourse.mybir as mybir
import concourse.tile as tile
from concourse import bacc
from concourse.bass_utils import run_bass_kernel_spmd
from concourse.masks import make_identity


P = 128
N_CORES = 8
B = 16384
B_LOC = B // N_CORES  # 2048 batch rows per core
D_IN = 784
KT = 7  # k tiles for layer-1 hi pass (784 -> 896 with packed lo-tail)
KLO = 6  # k tiles for layer-1 fp8 lo pass (the 768 full columns)
DP = KT * P  # 896
H = 2048
HS = H // P  # 16 hidden subtiles
NF = 512  # matmul free dim (one PSUM bank)
NB = B_LOC // NF  # 4 batch chunks
O = 10
KTAIL = D_IN - KLO * P  # 16-column tail (784 = 6*128 + 16)
LO_SCALE = float(2.0**14)  # lo pass: data scaled up, weights scaled down
W_LO = float(2.0**-14)  # exactly representable (normal) in fp8e5m2

F32 = mybir.dt.float32
F16 = mybir.dt.float16
F8 = mybir.dt.float8e4
F8E5 = mybir.dt.float8e5
AF = mybir.ActivationFunctionType
ALU = mybir.AluOpType
DR = mybir.MatmulPerfMode.DoubleRow


def _build():
    nc = bacc.Bacc(trn_type="TRN2")

    def din(name, shape):
        return nc.dram_tensor(name, shape, F32, kind="ExternalInput")

    x = din("x", [B_LOC, D_IN])
    w1 = din("w1", [H, D_IN])
    b1 = din("b1", [H])
    g1 = din("g1", [H])
    be1 = din("be1", [H])
    m1 = din("m1", [H])
    v1 = din("v1", [H])
    w2 = din("w2", [H, H])
    b2 = din("b2", [H])
    g2 = din("g2", [H])
    be2 = din("be2", [H])
    m2 = din("m2", [H])
    v2 = din("v2", [H])
    w3 = din("w3", [O, H])
    b3 = din("b3", [O])
    scale = din("scale", [1])
    out = nc.dram_tensor("out", [O, B_LOC], F32, kind="ExternalOutput")

    with ExitStack() as ctx:
        tc = ctx.enter_context(tile.TileContext(nc))
        consts = ctx.enter_context(tc.tile_pool(name="consts", bufs=1))
        big = ctx.enter_context(tc.tile_pool(name="big", bufs=1))
        stage = ctx.enter_context(tc.tile_pool(name="stage", bufs=2))
        wstage = ctx.enter_context(tc.tile_pool(name="wstage", bufs=2))
        psum = ctx.enter_context(tc.tile_pool(name="psum", bufs=1, space="PSUM"))

        # ---- identities for PE transposes ----
        id16 = consts.tile([P, P], F16, name="id16")
        make_identity(nc, id16)
        id32 = consts.tile([HS, HS], F32, name="id32")
        make_identity(nc, id32)

        # PE warm-up: dependency-free dummy matmuls on an uninitialized tile
        # (outputs never read). They run right after the engine barrier while
        # the first x/w1 tiles are still loading, so the HAM clock gate is
        # already at 8/8 when the real matmuls arrive.
        warm_in = consts.tile([P, NF], F16, name="warm_in")
        nc.gpsimd.memset(warm_in[:], 1.0)
        warm_ps = psum.tile([P, NF], F32, tag="mm", bufs=5, name="warm_ps")

        def warm(k):
            for _ in range(k):
                nc.tensor.matmul(
                    warm_ps[:], warm_in[:, :P], warm_in[:], start=True, stop=True
                )

        warm(14)

        # ---- persistent big tensors ----
        # h2b shares the xThi slot (xThi dead once L1 is done); xThi padded
        # to 8 k-tiles so the slot is h2b-sized.
        xThi = big.tile([P, 8, B_LOC], F16, tag="bigA", name="xThi")
        xTlo = big.tile([P, KLO, B_LOC], F8, tag="bigB", name="xTlo")
        w1bT = big.tile([P, KT, H], F16, tag="bigC", name="w1bT")
        w1lT = big.tile([P, KLO, H], F8E5, tag="bigD", name="w1lT")
        w2bT = big.tile([P, HS, H], F8, tag="bigE", name="w2bT")
        h1b = big.tile([P, HS, B_LOC], F8, tag="bigF", name="h1b")

        # ---- prep helpers ----
        # Layout transposes run on the PE, batched 4-per-PSUM-bank with wide
        # DVE drains (the 2^14 fp8 lo cast folds into the drain). Queue
        # assignment: x/w1/w2 streaming loads -> sync HWDGE (+ scalar for two
        # early x tiles), tiny/const loads -> gpsimd SWDGE.

        def tr_batch(dst, srctile, j0, cnt, chunk_sl, drain_scale=None):
            """Transpose `cnt` 128x128 fp16 blocks of srctile (block
            j0..j0+cnt-1) into one PSUM bank, then drain with a single wide
            DVE op into dst[:, j0:j0+cnt, chunk_sl]."""
            ps = psum.tile([P, 4 * P], F16, tag="tr", bufs=2, name="trps")
            for idx in range(cnt):
                j = j0 + idx
                nc.tensor.transpose(
                    ps[:, idx * P : (idx + 1) * P],
                    srctile[:, j * P : (j + 1) * P],
                    id16[:],
                )
            psv = ps[:, : cnt * P].rearrange("p (a b) -> p a b", b=P)
            dstv = dst[:, j0 : j0 + cnt, chunk_sl]
            if drain_scale is None:
                nc.vector.tensor_copy(dstv, psv)
            else:
                nc.vector.tensor_scalar(dstv, psv, drain_scale, None, ALU.mult)
            return ps

        def w1_prep(t):
            """Binarize w1 rows [128t:128t+128] -> fp16 with the k-tail
            duplicated at columns 784:800 (lo16-tail weights), PE-transpose
            into w1bT, then derive the +-2^-14 fp8e5 lo-pass weights."""
            w1sb = stage.tile([P, D_IN], F32, tag="w1f32", bufs=2, name="w1sb")
            nc.sync.dma_start(w1sb[:], w1[t * P : (t + 1) * P, :])
            w1bsb = stage.tile([P, DP], F16, tag="w1b16", bufs=2, name="w1bsb")
            nc.vector.memset(w1bsb[:, D_IN + KTAIL : DP], 0.0)
            nc.scalar.activation(w1bsb[:, :D_IN], w1sb[:], AF.Sign)
            nc.scalar.activation(
                w1bsb[:, D_IN : D_IN + KTAIL], w1sb[:, KLO * P : D_IN], AF.Sign
            )
            tsl = slice(t * P, (t + 1) * P)
            tr_batch(w1bT, w1bsb, 0, 4, tsl)
            tr_batch(w1bT, w1bsb, 4, 3, tsl)
            nc.vector.tensor_scalar(
                w1lT[:, :, tsl], w1bT[:, :KLO, tsl], W_LO, None, ALU.mult
            )

        def x_prep(t):
            """DMA a [128, 784] fp32 x tile, split hi (fp16, with the exact
            fp16 lo-tail packed at columns 784:800) / lo (fp16, cast to
            scaled fp8e4 at the transpose drain), then PE-transpose 7 + 6
            fp16 blocks."""
            xsb = stage.tile([P, D_IN], F32, tag="xf32", bufs=2, name="xsb")
            ldq = nc.scalar if t in (2, 3) else nc.sync
            ldq.dma_start(xsb[:], x[t * P : (t + 1) * P, :])
            hl16 = stage.tile([P, DP], F16, tag="xhi16", bufs=2, name="hl16")
            nc.vector.memset(hl16[:, D_IN + KTAIL : DP], 0.0)
            if t in (2, 3):
                nc.scalar.copy(hl16[:, :D_IN], xsb[:])
            else:
                nc.vector.tensor_copy(hl16[:, :D_IN], xsb[:])
            nc.vector.tensor_tensor(
                hl16[:, D_IN : D_IN + KTAIL],
                xsb[:, KLO * P : D_IN],
                hl16[:, KLO * P : D_IN],
                ALU.subtract,
            )
            t16 = stage.tile([P, KLO * P], F16, tag="xlo16", bufs=2, name="t16")
            nc.vector.tensor_tensor(
                t16[:], xsb[:, : KLO * P], hl16[:, : KLO * P], ALU.subtract
            )
            tsl = slice(t * P, (t + 1) * P)
            tr_batch(xThi, hl16, 0, 4, tsl)
            tr_batch(xThi, hl16, 4, 3, tsl)
            tr_batch(xTlo, t16, 0, 4, tsl, drain_scale=LO_SCALE)
            tr_batch(xTlo, t16, 4, 2, tsl, drain_scale=LO_SCALE)

        def w2_prep(o):
            """Binarize w2 rows [128o:128o+128] -> fp16, PE-transpose, cast
            to fp8 into the DoubleRow-packed w2bT."""
            w2b = wstage.tile([P, H], F16, tag="w2b16", bufs=1, name="w2b")
            for half in range(2):
                hsl2 = slice(half * (H // 2), (half + 1) * (H // 2))
                w2sb = wstage.tile([P, H // 2], F32, tag="w2f32", name="w2sb")
                nc.scalar.dma_start(w2sb[:], w2[o * P : (o + 1) * P, hsl2])
                nc.scalar.activation(w2b[:, hsl2], w2sb[:], AF.Sign)
            osl2 = slice(o * P, (o + 1) * P)
            for g in range(4):
                tr_batch(w2bT, w2b, 4 * g, 4, osl2)

        def l1_mm(n, h):
            nsl = slice(n * NF, (n + 1) * NF)
            pmm = psum.tile([P, NF], F32, tag="mm", bufs=5, name="pmm")
            hsl = slice(h * P, (h + 1) * P)
            for k in range(KT):
                nc.tensor.matmul(
                    pmm[:], w1bT[:, k, hsl], xThi[:, k, nsl], start=(k == 0), stop=False
                )
            for kk in range(KLO // 2):
                ksl = slice(2 * kk, 2 * kk + 2)
                nc.tensor.matmul(
                    pmm[:],
                    w1lT[:, ksl, hsl],
                    xTlo[:, ksl, nsl],
                    start=False,
                    stop=(kk == KLO // 2 - 1),
                    perf_mode=DR,
                )
            nc.scalar.activation(
                h1b[:, h, nsl],
                pmm[:],
                AF.Sign,
                bias=c1[:, h : h + 1],
                scale=inv1[:, h : h + 1],
            )

        def const_setup():
            # ---- per-hidden-unit BN constants, laid out [p, subtile] ----
            def vec_sb(handle, name):
                tmp = stage.tile([HS, P], F32, tag="vtmp", bufs=2, name="vtmp")
                nc.gpsimd.dma_start(tmp[:], handle[:].rearrange("(s p) -> s p", p=P))
                ps = psum.tile([P, HS], F32, tag="tr", bufs=2, name="vtps")
                nc.tensor.transpose(ps[:], tmp[:], id32[:])
                t = consts.tile([P, HS], F32, name=name)
                nc.vector.tensor_copy(t[:], ps[:])
                return t

            b1s = vec_sb(b1, "b1s")
            g1s = vec_sb(g1, "g1s")
            be1s = vec_sb(be1, "be1s")
            m1s = vec_sb(m1, "m1s")
            v1s = vec_sb(v1, "v1s")
            b2s = vec_sb(b2, "b2s")
            g2s = vec_sb(g2, "g2s")
            be2s = vec_sb(be2, "be2s")
            m2s = vec_sb(m2, "m2s")
            v2s = vec_sb(v2, "v2s")

            def bn_fold(gs, bes, ms, bs, vs, tag):
                inv = consts.tile([P, HS], F32, name=f"inv{tag}")
                c = consts.tile([P, HS], F32, name=f"c{tag}")
                nc.vector.tensor_scalar_add(inv, vs, 1e-5)
                nc.scalar.activation(inv, inv, AF.Sqrt)
                nc.vector.reciprocal(inv, inv)
                nc.vector.tensor_mul(inv, gs, inv)
                nc.vector.tensor_sub(c, bs, ms)
                nc.vector.tensor_mul(c, c, inv)
                nc.vector.tensor_add(c, c, bes)
                return inv, c

            inv1, c1 = bn_fold(g1s, be1s, m1s, b1s, v1s, "1")
            inv2, c2 = bn_fold(g2s, be2s, m2s, b2s, v2s, "2")

            # b3 and scale broadcast onto 10 partitions
            b3sb = consts.tile([O, 1], F32, name="b3sb")
            nc.gpsimd.dma_start(b3sb[:], b3[:].rearrange("(o u) -> o u", u=1))
            s10 = consts.tile([O, 1], F32, name="s10")
            for i in range(O):
                nc.gpsimd.dma_start(
                    s10[i : i + 1, :], scale[:].rearrange("(s u) -> s u", u=1)
                )
            return inv1, c1, inv2, c2, b3sb, s10

        # ---- main pipeline over batch chunks, software-pipelined prep ----
        for t in range(4):
            x_prep(t)
            warm(3)
        for t in range(4):
            w1_prep(t)
            warm(2)
        inv1, c1, inv2, c2, b3sb, s10 = const_setup()
        for n in range(NB):
            for h in range(HS):
                if n == 0 and h + 4 < HS:
                    w1_prep(h + 4)
                if 4 <= h < 8 and n + 1 < NB:
                    x_prep(4 * (n + 1) + (h - 4))
                if h % 4 == 3:
                    w2_prep(4 * n + h // 4)
                l1_mm(n, h)

        # ---- w3 prep (chunked to keep SBUF small) ----
        w3bT = consts.tile([P, HS, 16], F8, name="w3bT")
        for ks in range(HS):
            ksl = slice(ks * P, (ks + 1) * P)
            w3sb = stage.tile([O, P], F32, tag="w3f32", name="w3sb")
            nc.gpsimd.dma_start(w3sb[:], w3[:, ksl])
            w3b = stage.tile([O, P], F16, tag="w3b16", name="w3b")
            nc.scalar.activation(w3b[:], w3sb[:], AF.Sign)
            ps = psum.tile([P, 16], F16, tag="tr", bufs=2, name="trps3")
            nc.tensor.transpose(ps[:, :O], w3b[:], id16[:O, :O])
            nc.vector.tensor_copy(w3bT[:, ks, :O], ps[:, :O])

        # ---- layer 2 with layer-3 DoubleRow matmuls interleaved (lagged
        # two o-tiles so the Sign drains stay off the PE critical path) ----
        h2b = big.tile([P, HS, B_LOC], F8, tag="bigA", name="h2b")
        for n in range(NB):
            nsl = slice(n * NF, (n + 1) * NF)
            p3 = psum.tile([P, NF], F32, tag="l3", bufs=1, name="p3")

            def l3_pair(kk, n=n, nsl=nsl, p3=p3):
                ksl = slice(2 * kk, 2 * kk + 2)
                nc.tensor.matmul(
                    p3[:O, :],
                    w3bT[:, ksl, :O],
                    h2b[:, ksl, nsl],
                    start=(kk == 0),
                    stop=(kk == HS // 2 - 1),
                    perf_mode=DR,
                )

            for o in range(HS):
                if o >= 3 and o % 2 == 1:
                    l3_pair((o - 3) // 2)
                osl = slice(o * P, (o + 1) * P)
                pmm = psum.tile([P, NF], F32, tag="mm", bufs=5, name="pmm")
                for kk in range(HS // 2):
                    ksl = slice(2 * kk, 2 * kk + 2)
                    nc.tensor.matmul(
                        pmm[:],
                        w2bT[:, ksl, osl],
                        h1b[:, ksl, nsl],
                        start=(kk == 0),
                        stop=(kk == HS // 2 - 1),
                        perf_mode=DR,
                    )
                nc.scalar.activation(
                    h2b[:, o, nsl],
                    pmm[:],
                    AF.Sign,
                    bias=c2[:, o : o + 1],
                    scale=inv2[:, o : o + 1],
                )
            l3_pair(7)
            outsb = stage.tile([O, NF], F32, tag="outsb", name="outsb")
            nc.vector.tensor_scalar(
                outsb[:], p3[:O, :], b3sb[:], s10[:], ALU.add, ALU.mult
            )
            nc.sync.dma_start(out[:, nsl], outsb[:])

    nc.finalize()
    return nc


_CACHE = {}


def _get_nc():
    if "nc" not in _CACHE:
        _CACHE["nc"] = _build()
    return _CACHE["nc"]


def _in_maps(x, w1, b1, g1, be1, m1, v1, w2, b2, g2, be2, m2, v2, w3, b3, scale):
    f = lambda a: np.ascontiguousarray(np.asarray(a, dtype=np.float32))
    x2 = f(x).reshape(B, D_IN)
    base = {
        "w1": f(w1),
        "b1": f(b1),
        "g1": f(g1),
        "be1": f(be1),
        "m1": f(m1),
        "v1": f(v1),
        "w2": f(w2),
        "b2": f(b2),
        "g2": f(g2),
        "be2": f(be2),
        "m2": f(m2),
        "v2": f(v2),
        "w3": f(w3),
        "b3": f(b3),
        "scale": f(scale).reshape(1),
    }
    maps = []
    for c in range(N_CORES):
        m = dict(base)
        m["x"] = np.ascontiguousarray(x2[c * B_LOC : (c + 1) * B_LOC])
        maps.append(m)
    return maps


def _ensure_ntff_hook():
    """The agent image's antenv package lacks axon_hooks; synthesize it so
    run_bass_kernel_spmd's trace path can reach the axon NTFF profiler."""
    import sys
    import types

    if "antenv.axon_hooks" in sys.modules:
        return
    mod = types.ModuleType("antenv.axon_hooks")
    mod._hook = None

    def set_axon_ntff_profile_hook(h):
        mod._hook = h

    def get_axon_ntff_profile_hook():
        return mod._hook

    mod.set_axon_ntff_profile_hook = set_axon_ntff_profile_hook
    mod.get_axon_ntff_profile_hook = get_axon_ntff_profile_hook
    sys.modules["antenv.axon_hooks"] = mod
    import antenv

    antenv.axon_hooks = mod
    try:
        from trn_agent_boot.trn_boot import _ntff_profile_via_ctypes

        mod._hook = _ntff_profile_via_ctypes("/opt/axon/libaxon_pjrt.so")
    except Exception as e:
        print(f"ntff hook unavailable: {e}")


def run(trace=False, **inputs):
    if trace:
        _ensure_ntff_hook()
    nc = _get_nc()
    res = run_bass_kernel_spmd(
        nc, _in_maps(**inputs), core_ids=list(range(N_CORES)), trace=trace
    )
    outs = [r["out"] for r in res.results]
    full = np.concatenate([o.T for o in outs], axis=0).astype(np.float32)
    return full, res


def kernel(**inputs):
    return run(trace=False, **inputs)[0]


# revision 42
# speedup vs baseline: 1.0180x; 1.0180x over previous
"""Trainium2 Bass kernel for the BNN FASHION FC problem.

Network (per reference):
  h = x.reshape(B, 784)
  L1: h @ binarize(w1).T + b1 -> BN -> clip -> binarize     [B, 2048]
  L2: h @ binarize(w2).T + b2 -> BN -> clip -> binarize     [B, 2048]
  L3: (h @ binarize(w3).T + b3) * scale                     [B, 10]

Strategy (8 NeuronCores, data-parallel over batch, weights replicated):
  - Everything is computed with the hidden index on SBUF/PSUM partitions,
    so batchnorm affine + sign folds into one ScalarE activation (Sign
    with per-partition scale/bias) and layer N's output lands exactly in
    the [k=hidden, free=batch] layout layer N+1 needs.
  - L1 splits x = hi + lo. hi runs in fp16 over 7 k-tiles, with the
    packed tail tile carrying hi rows 0:16 and an exact fp16 lo-tail in
    rows 16:32 (both against +-1 fp16 weights).  lo for the six full
    k-tiles runs in fp8e4m3 scaled by 2^14 against +-2^-14 fp8e5m2
    weights in DoubleRow mode (3 matmuls).  10 matmuls/tile total vs 13
    for the exact fp16 hi/lo scheme; final rel err ~1.4e-2 (gate 2e-2).
  - L2/L3 operands are all +-1: exact in fp8e4, run with DoubleRow.
  - binarize(clip(y)) == binarize(y), so clip is dropped.
  - BN folding: y = mm*inv + c with inv = g/sqrt(v+eps), c = (b-m)*inv + be.
  - L3 DoubleRow matmuls are interleaved into the L2 o-loop (lagged so
    the Sign drains stay off the PE critical path), shortening the tail.

Output per core is [10, 2048] (hidden-major); the host transposes and
concatenates to the full [16384, 10].
"""

import numpy as np
from contextlib import ExitStack

try:
    import concourse.bass as bass
except ImportError:  # staged repo location
    import sys

    sys.path.insert(0, "/opt/trn_rl_repo")
    import concourse.bass as bass

import concourse.mybir as mybir
import concourse.tile as tile
from concourse import bacc
from concourse.bass_utils import run_bass_kernel_spmd
from concourse.masks import make_identity


P = 128
N_CORES = 8
B = 16384
B_LOC = B // N_CORES  # 2048 batch rows per core
D_IN = 784
KT = 7  # k tiles for layer-1 hi pass (784 -> 896 with packed lo-tail)
KLO = 6  # k tiles for layer-1 fp8 lo pass (the 768 full columns)
DP = KT * P  # 896
H = 2048
HS = H // P  # 16 hidden subtiles
NF = 512  # matmul free dim (one PSUM bank)
NB = B_LOC // NF  # 4 batch chunks
O = 10
KTAIL = D_IN - KLO * P  # 16-column tail (784 = 6*128 + 16)
LO_SCALE = float(2.0**14)  # lo pass: data scaled up, weights scaled down
W_LO = float(2.0**-14)  # exactly representable (normal) in fp8e5m2

F32 = mybir.dt.float32
F16 = mybir.dt.float16
F8 = mybir.dt.float8e4
F8E5 = mybir.dt.float8e5
AF = mybir.ActivationFunctionType
ALU = mybir.AluOpType
DR = mybir.MatmulPerfMode.DoubleRow


def _build():
    nc = bacc.Bacc(trn_type="TRN2")

    def din(name, shape):
        return nc.dram_tensor(name, shape, F32, kind="ExternalInput")

    x = din("x", [B_LOC, D_IN])
    w1 = din("w1", [H, D_IN])
    b1 = din("b1", [H])
    g1 = din("g1", [H])
    be1 = din("be1", [H])
    m1 = din("m1", [H])
    v1 = din("v1", [H])
    w2 = din("w2", [H, H])
    b2 = din("b2", [H])
    g2 = din("g2", [H])
    be2 = din("be2", [H])
    m2 = din("m2", [H])
    v2 = din("v2", [H])
    w3 = din("w3", [O, H])
    b3 = din("b3", [O])
    scale = din("scale", [1])
    out = nc.dram_tensor("out", [O, B_LOC], F32, kind="ExternalOutput")

    with ExitStack() as ctx:
        tc = ctx.enter_context(tile.TileContext(nc))
        consts = ctx.enter_context(tc.tile_pool(name="consts", bufs=1))
        big = ctx.enter_context(tc.tile_pool(name="big", bufs=1))
        stage = ctx.enter_context(tc.tile_pool(name="stage", bufs=2))
        wstage = ctx.enter_context(tc.tile_pool(name="wstage", bufs=2))
        psum = ctx.enter_context(tc.tile_pool(name="psum", bufs=1, space="PSUM"))

        # ---- identities for PE transposes ----
        id16 = consts.tile([P, P], F16, name="id16")
        make_identity(nc, id16)
        id32 = consts.tile([HS, HS], F32, name="id32")
        make_identity(nc, id32)

        # PE warm-up: dependency-free dummy matmuls on an uninitialized tile
        # (outputs never read). They run right after the engine barrier while
        # the first x/w1 tiles are still loading, so the HAM clock gate is
        # already at 8/8 when the real matmuls arrive.
        warm_in = consts.tile([P, NF], F16, name="warm_in")
        nc.gpsimd.memset(warm_in[:], 1.0)
        warm_ps = psum.tile([P, NF], F32, tag="mm", bufs=5, name="warm_ps")

        def warm(k):
            for _ in range(k):
                nc.tensor.matmul(
                    warm_ps[:], warm_in[:, :P], warm_in[:], start=True, stop=True
                )

        warm(14)

        # ---- persistent big tensors ----
        # h2b shares the xThi slot (xThi dead once L1 is done); xThi padded
        # to 8 k-tiles so the slot is h2b-sized.
        xThi = big.tile([P, 8, B_LOC], F16, tag="bigA", name="xThi")
        xTlo = big.tile([P, KLO, B_LOC], F8, tag="bigB", name="xTlo")
        w1bT = big.tile([P, KT, H], F16, tag="bigC", name="w1bT")
        w1lT = big.tile([P, KLO, H], F8E5, tag="bigD", name="w1lT")
        w2bT = big.tile([P, HS, H], F8, tag="bigE", name="w2bT")
        h1b = big.tile([P, HS, B_LOC], F8, tag="bigF", name="h1b")

        # ---- prep helpers ----
        # Layout transposes run on the PE, batched 4-per-PSUM-bank with wide
        # DVE drains (the 2^14 fp8 lo cast folds into the drain). Queue
        # assignment: x/w1/w2 streaming loads -> sync HWDGE (+ scalar for two
        # early x tiles), tiny/const loads -> gpsimd SWDGE.

        def tr_batch(dst, srctile, j0, cnt, chunk_sl, drain_scale=None):
            """Transpose `cnt` 128x128 fp16 blocks of srctile (block
            j0..j0+cnt-1) into one PSUM bank, then drain with a single wide
            DVE op into dst[:, j0:j0+cnt, chunk_sl]."""
            ps = psum.tile([P, 4 * P], F16, tag="tr", bufs=2, name="trps")
            for idx in range(cnt):
                j = j0 + idx
                nc.tensor.transpose(
                    ps[:, idx * P : (idx + 1) * P],
                    srctile[:, j * P : (j + 1) * P],
                    id16[:],
                )
            psv = ps[:, : cnt * P].rearrange("p (a b) -> p a b", b=P)
            dstv = dst[:, j0 : j0 + cnt, chunk_sl]
            if drain_scale is None:
                nc.vector.tensor_copy(dstv, psv)
            else:
                nc.vector.tensor_scalar(dstv, psv, drain_scale, None, ALU.mult)
            return ps

        def w1_prep(t):
            """Binarize w1 rows [128t:128t+128] -> fp16 with the k-tail
            duplicated at columns 784:800 (lo16-tail weights), PE-transpose
            into w1bT, then derive the +-2^-14 fp8e5 lo-pass weights."""
            w1sb = stage.tile([P, D_IN], F32, tag="w1f32", bufs=2, name="w1sb")
            nc.sync.dma_start(w1sb[:], w1[t * P : (t + 1) * P, :])
            w1bsb = stage.tile([P, DP], F16, tag="w1b16", bufs=2, name="w1bsb")
            nc.vector.memset(w1bsb[:, D_IN + KTAIL : DP], 0.0)
            nc.scalar.activation(w1bsb[:, :D_IN], w1sb[:], AF.Sign)
            nc.scalar.activation(
                w1bsb[:, D_IN : D_IN + KTAIL], w1sb[:, KLO * P : D_IN], AF.Sign
            )
            tsl = slice(t * P, (t + 1) * P)
            tr_batch(w1bT, w1bsb, 0, 4, tsl)
            tr_batch(w1bT, w1bsb, 4, 3, tsl)
            nc.vector.tensor_scalar(
                w1lT[:, :, tsl], w1bT[:, :KLO, tsl], W_LO, None, ALU.mult
            )

        def x_prep(t):
            """DMA a [128, 784] fp32 x tile, split hi (fp16, with the exact
            fp16 lo-tail packed at columns 784:800) / lo (fp16, cast to
            scaled fp8e4 at the transpose drain), then PE-transpose 7 + 6
            fp16 blocks."""
            xsb = stage.tile([P, D_IN], F32, tag="xf32", bufs=2, name="xsb")
            ldq = nc.scalar if t in (2, 3) else nc.sync
            ldq.dma_start(xsb[:], x[t * P : (t + 1) * P, :])
            hl16 = stage.tile([P, DP], F16, tag="xhi16", bufs=2, name="hl16")
            nc.vector.memset(hl16[:, D_IN + KTAIL : DP], 0.0)
            if t in (2, 3):
                nc.scalar.copy(hl16[:, :D_IN], xsb[:])
            else:
                nc.vector.tensor_copy(hl16[:, :D_IN], xsb[:])
            nc.vector.tensor_tensor(
                hl16[:, D_IN : D_IN + KTAIL],
                xsb[:, KLO * P : D_IN],
                hl16[:, KLO * P : D_IN],
                ALU.subtract,
            )
            t16 = stage.tile([P, KLO * P], F16, tag="xlo16", bufs=2, name="t16")
            nc.vector.tensor_tensor(
                t16[:], xsb[:, : KLO * P], hl16[:, : KLO * P], ALU.subtract
            )
            tsl = slice(t * P, (t + 1) * P)
            tr_batch(xThi, hl16, 0, 4, tsl)
            tr_batch(xThi, hl16, 4, 3, tsl)
            tr_batch(xTlo, t16, 0, 4, tsl, drain_scale=LO_SCALE)
            tr_batch(xTlo, t16, 4, 2, tsl, drain_scale=LO_SCALE)

        def w2_prep(o):
            """Binarize w2 rows [128o:128o+128] -> fp16, PE-transpose, cast
            to fp8 into the DoubleRow-packed w2bT."""
            w2b = wstage.tile([P, H], F16, tag="w2b16", bufs=1, name="w2b")
            for half in range(2):
                hsl2 = slice(half * (H // 2), (half + 1) * (H // 2))
                w2sb = wstage.tile([P, H // 2], F32, tag="w2f32", name="w2sb")
                nc.sync.dma_start(w2sb[:], w2[o * P : (o + 1) * P, hsl2])
                nc.scalar.activation(w2b[:, hsl2], w2sb[:], AF.Sign)
            osl2 = slice(o * P, (o + 1) * P)
            for g in range(4):
                tr_batch(w2bT, w2b, 4 * g, 4, osl2)

        def l1_mm(n, h):
            nsl = slice(n * NF, (n + 1) * NF)
            pmm = psum.tile([P, NF], F32, tag="mm", bufs=5, name="pmm")
            hsl = slice(h * P, (h + 1) * P)
            for k in range(KT):
                nc.tensor.matmul(
                    pmm[:], w1bT[:, k, hsl], xThi[:, k, nsl], start=(k == 0), stop=False
                )
            for kk in range(KLO // 2):
                ksl = slice(2 * kk, 2 * kk + 2)
                nc.tensor.matmul(
                    pmm[:],
                    w1lT[:, ksl, hsl],
                    xTlo[:, ksl, nsl],
                    start=False,
                    stop=(kk == KLO // 2 - 1),
                    perf_mode=DR,
                )
            nc.scalar.activation(
                h1b[:, h, nsl],
                pmm[:],
                AF.Sign,
                bias=c1[:, h : h + 1],
                scale=inv1[:, h : h + 1],
            )

        def const_setup():
            # ---- per-hidden-unit BN constants, laid out [p, subtile] ----
            def vec_sb(handle, name):
                tmp = stage.tile([HS, P], F32, tag="vtmp", bufs=2, name="vtmp")
                nc.gpsimd.dma_start(tmp[:], handle[:].rearrange("(s p) -> s p", p=P))
                ps = psum.tile([P, HS], F32, tag="tr", bufs=2, name="vtps")
                nc.tensor.transpose(ps[:], tmp[:], id32[:])
                t = consts.tile([P, HS], F32, name=name)
                nc.vector.tensor_copy(t[:], ps[:])
                return t

            b1s = vec_sb(b1, "b1s")
            g1s = vec_sb(g1, "g1s")
            be1s = vec_sb(be1, "be1s")
            m1s = vec_sb(m1, "m1s")
            v1s = vec_sb(v1, "v1s")
            b2s = vec_sb(b2, "b2s")
            g2s = vec_sb(g2, "g2s")
            be2s = vec_sb(be2, "be2s")
            m2s = vec_sb(m2, "m2s")
            v2s = vec_sb(v2, "v2s")

            def bn_fold(gs, bes, ms, bs, vs, tag):
                inv = consts.tile([P, HS], F32, name=f"inv{tag}")
                c = consts.tile([P, HS], F32, name=f"c{tag}")
                nc.vector.tensor_scalar_add(inv, vs, 1e-5)
                nc.scalar.activation(inv, inv, AF.Sqrt)
                nc.vector.reciprocal(inv, inv)
                nc.vector.tensor_mul(inv, gs, inv)
                nc.vector.tensor_sub(c, bs, ms)
                nc.vector.tensor_mul(c, c, inv)
                nc.vector.tensor_add(c, c, bes)
                return inv, c

            inv1, c1 = bn_fold(g1s, be1s, m1s, b1s, v1s, "1")
            inv2, c2 = bn_fold(g2s, be2s, m2s, b2s, v2s, "2")

            # b3 and scale broadcast onto 10 partitions
            b3sb = consts.tile([O, 1], F32, name="b3sb")
            nc.gpsimd.dma_start(b3sb[:], b3[:].rearrange("(o u) -> o u", u=1))
            s10 = consts.tile([O, 1], F32, name="s10")
            for i in range(O):
                nc.gpsimd.dma_start(
                    s10[i : i + 1, :], scale[:].rearrange("(s u) -> s u", u=1)
                )
            return inv1, c1, inv2, c2, b3sb, s10

        # ---- main pipeline over batch chunks, software-pipelined prep ----
        for t in range(4):
            x_prep(t)
            warm(3)
        for t in range(4):
            w1_prep(t)
            warm(2)
        inv1, c1, inv2, c2, b3sb, s10 = const_setup()
        for n in range(NB):
            for h in range(HS):
                if n == 0 and h + 4 < HS:
                    w1_prep(h + 4)
                if 4 <= h < 8 and n + 1 < NB:
                    x_prep(4 * (n + 1) + (h - 4))
                if n >= 1 and h % 2 == 1 and 8 * (n - 1) + h // 2 < HS:
                    w2_prep(8 * (n - 1) + h // 2)
                l1_mm(n, h)

        # ---- w3 prep (chunked to keep SBUF small) ----
        w3bT = consts.tile([P, HS, 16], F8, name="w3bT")
        for ks in range(HS):
            ksl = slice(ks * P, (ks + 1) * P)
            w3sb = stage.tile([O, P], F32, tag="w3f32", name="w3sb")
            nc.gpsimd.dma_start(w3sb[:], w3[:, ksl])
            w3b = stage.tile([O, P], F16, tag="w3b16", name="w3b")
            nc.scalar.activation(w3b[:], w3sb[:], AF.Sign)
            ps = psum.tile([P, 16], F16, tag="tr", bufs=2, name="trps3")
            nc.tensor.transpose(ps[:, :O], w3b[:], id16[:O, :O])
            nc.vector.tensor_copy(w3bT[:, ks, :O], ps[:, :O])

        # ---- layer 2 with layer-3 DoubleRow matmuls interleaved (lagged
        # two o-tiles so the Sign drains stay off the PE critical path) ----
        h2b = big.tile([P, HS, B_LOC], F8, tag="bigA", name="h2b")
        for n in range(NB):
            nsl = slice(n * NF, (n + 1) * NF)
            p3 = psum.tile([P, NF], F32, tag="l3", bufs=1, name="p3")

            def l3_pair(kk, n=n, nsl=nsl, p3=p3):
                ksl = slice(2 * kk, 2 * kk + 2)
                nc.tensor.matmul(
                    p3[:O, :],
                    w3bT[:, ksl, :O],
                    h2b[:, ksl, nsl],
                    start=(kk == 0),
                    stop=(kk == HS // 2 - 1),
                    perf_mode=DR,
                )

            for o in range(HS):
                if o >= 3 and o % 2 == 1:
                    l3_pair((o - 3) // 2)
                osl = slice(o * P, (o + 1) * P)
                pmm = psum.tile([P, NF], F32, tag="mm", bufs=5, name="pmm")
                for kk in range(HS // 2):
                    ksl = slice(2 * kk, 2 * kk + 2)
                    nc.tensor.matmul(
                        pmm[:],
                        w2bT[:, ksl, osl],
                        h1b[:, ksl, nsl],
                        start=(kk == 0),
                        stop=(kk == HS // 2 - 1),
                        perf_mode=DR,
                    )
                nc.scalar.activation(
                    h2b[:, o, nsl],
                    pmm[:],
                    AF.Sign,
                    bias=c2[:, o : o + 1],
                    scale=inv2[:, o : o + 1],
                )
            l3_pair(7)
            outsb = stage.tile([O, NF], F32, tag="outsb", name="outsb")
            nc.vector.tensor_scalar(
                outsb[:], p3[:O, :], b3sb[:], s10[:], ALU.add, ALU.mult
            )
            nc.sync.dma_start(out[:, nsl], outsb[:])

    nc.finalize()
    return nc


_CACHE = {}


def _get_nc():
    if "nc" not in _CACHE:
        _CACHE["nc"] = _build()
    return _CACHE["nc"]


def _in_maps(x, w1, b1, g1, be1, m1, v1, w2, b2, g2, be2, m2, v2, w3, b3, scale):
    f = lambda a: np.ascontiguousarray(np.asarray(a, dtype=np.float32))
    x2 = f(x).reshape(B, D_IN)
    base = {
        "w1": f(w1),
        "b1": f(b1),
        "g1": f(g1),
        "be1": f(be1),
        "m1": f(m1),
        "v1": f(v1),
        "w2": f(w2),
        "b2": f(b2),
        "g2": f(g2),
        "be2": f(be2),
        "m2": f(m2),
        "v2": f(v2),
        "w3": f(w3),
        "b3": f(b3),
        "scale": f(scale).reshape(1),
    }
    maps = []
    for c in range(N_CORES):
        m = dict(base)
        m["x"] = np.ascontiguousarray(x2[c * B_LOC : (c + 1) * B_LOC])
        maps.append(m)
    return maps


def _ensure_ntff_hook():
    """The agent image's antenv package lacks axon_hooks; synthesize it so
    run_bass_kernel_spmd's trace path can reach the axon NTFF profiler."""
    import sys
    import types

    if "antenv.axon_hooks" in sys.modules:
        return
    mod = types.ModuleType("antenv.axon_hooks")
    mod._hook = None

    def set_axon_ntff_profile_hook(h):
        mod._hook = h

    def get_axon_ntff_profile_hook():
        return mod._hook

    mod.set_axon_ntff_profile_hook = set_axon_ntff_profile_hook
    mod.get_axon_ntff_profile_hook = get_axon_ntff_profile_hook
    sys.modules["antenv.axon_hooks"] = mod
    import antenv

    antenv.axon_hooks = mod
    try:
        from trn_agent_boot.trn_boot import _ntff_profile_via_ctypes

        mod._hook = _ntff_profile_via_ctypes("/opt/axon/libaxon_pjrt.so")
    except Exception as e:
        print(f"ntff hook unavailable: {e}")


def run(trace=False, **inputs):
    if trace:
        _ensure_ntff_hook()
    nc = _get_nc()
    res = run_bass_kernel_spmd(
        nc, _in_maps(**inputs), core_ids=list(range(N_CORES)), trace=trace
    )
    outs = [r["out"] for r in res.results]
    full = np.concatenate([o.T for o in outs], axis=0).astype(np.float32)
    return full, res


def kernel(**inputs):
    return run(trace=False, **inputs)[0]
